# revision 3
# baseline (speedup 1.0000x reference)
"""BinaryLinear (XNOR-Net style) Trainium2 kernel.

y = x @ (sign(W) * alpha)^T + bias,  alpha = mean(|W|, axis=1)

Strategy: data-parallel over the 16384-token dim across 8 NeuronCores.
Host folds the weight transform: signs are exactly representable in bf16,
so each core runs a bf16 matmul  y_shard^T[o, n] = sum_i sign(W)[o,i] *
x[n,i]  with fp32 PSUM accumulation, then applies the fp32 per-row scale
alpha[o] and bias[o] on the Scalar engine.  Host gathers/transposes back.
"""

import numpy as np
import ml_dtypes

N_CORES = 8
N_TOK = 16384
K = 4096  # in_features (contraction)
O = 4096  # out_features
P = 128
N_SHARD = N_TOK // N_CORES  # 2048 tokens per core
KO = K // P  # 32 contraction tiles
OT = O // P  # 32 output-feature tiles
NT = 512  # matmul moving free dim (one fp32 PSUM bank)
N_NT = N_SHARD // NT  # 4

_NC_CACHE = {}


def _build(n_shard=N_SHARD, ko=KO, ot=OT, nt=NT):
    import concourse.mybir as mybir
    import concourse.tile as tile
    from concourse import bacc

    bf16 = mybir.dt.bfloat16
    f32 = mybir.dt.float32
    n_nt = n_shard // nt

    nc = bacc.Bacc("TRN2", target_bir_lowering=False, debug=False, num_devices=N_CORES)
    xt_d = nc.dram_tensor("xt", [ko, P, n_shard], bf16, kind="ExternalInput")
    st_d = nc.dram_tensor("st", [ot, P, ko, P], bf16, kind="ExternalInput")
    al_d = nc.dram_tensor("alpha", [P, ot], f32, kind="ExternalInput")
    bi_d = nc.dram_tensor("bias", [P, ot], f32, kind="ExternalInput")
    yt_d = nc.dram_tensor("yt", [ot, P, n_shard], f32, kind="ExternalOutput")

    with tile.TileContext(nc) as tc:
        with (
            tc.tile_pool(name="xpool", bufs=1) as xpool,
            tc.tile_pool(name="spool", bufs=3) as spool,
            tc.tile_pool(name="opool", bufs=6) as opool,
            tc.tile_pool(name="cpool", bufs=1) as cpool,
            tc.tile_pool(name="psum", bufs=8, space="PSUM") as pp,
        ):
            al_t = cpool.tile([P, ot], f32)
            nc.sync.dma_start(al_t[:], al_d[:])
            bi_t = cpool.tile([P, ot], f32)
            nc.sync.dma_start(bi_t[:], bi_d[:])

            # x^T shard stays resident in SBUF: [128, ko, n_shard] bf16.
            xt_t = xpool.tile([P, ko, n_shard], bf16)
            for k in range(ko):
                nc.sync.dma_start(xt_t[:, k, :], xt_d[k])

            for o in range(ot):
                s_t = spool.tile([P, ko, P], bf16)
                nc.sync.dma_start(s_t[:], st_d[o])
                for n in range(n_nt):
                    ps = pp.tile([P, nt], f32)
                    for k in range(ko):
                        nc.tensor.matmul(
                            ps[:],
                            s_t[:, k, :],
                            xt_t[:, k, n * nt : (n + 1) * nt],
                            start=(k == 0),
                            stop=(k == ko - 1),
                        )
                    ob = opool.tile([P, nt], f32)
                    nc.scalar.activation(
                        ob[:],
                        ps[:],
                        mybir.ActivationFunctionType.Identity,
                        bias=bi_t[:, o : o + 1],
                        scale=al_t[:, o : o + 1],
                    )
                    nc.sync.dma_start(yt_d[o, :, n * nt : (n + 1) * nt], ob[:])
    nc.compile()
    return nc


def get_nc():
    if "nc" not in _NC_CACHE:
        _NC_CACHE["nc"] = _build()
    return _NC_CACHE["nc"]


def prep_inputs(x, weight, bias):
    """Host-side shard + layout prep. Returns in_maps for the 8 cores."""
    bf16 = ml_dtypes.bfloat16
    w = np.asarray(weight, dtype=np.float32)
    alpha = np.abs(w).mean(axis=1, dtype=np.float32).astype(np.float32)  # [O]
    s = np.sign(w).astype(bf16)  # [O, K], exactly +-1 (or 0)
    # layout (ot, p=k%128, ko, oi): 8KB contiguous per partition per o-tile
    st = np.ascontiguousarray(s.reshape(OT, P, KO, P).transpose(0, 3, 2, 1))
    al = np.ascontiguousarray(alpha.reshape(OT, P).T)
    bi = np.ascontiguousarray(np.asarray(bias, dtype=np.float32).reshape(OT, P).T)

    in_maps = []
    for c in range(N_CORES):
        xc = np.asarray(x[c * N_SHARD : (c + 1) * N_SHARD], dtype=np.float32)
        xt = np.ascontiguousarray(xc.T).astype(bf16).reshape(KO, P, N_SHARD)
        in_maps.append({"xt": xt, "st": st, "alpha": al, "bias": bi})
    return in_maps


def gather_output(results):
    outs = []
    for c in range(N_CORES):
        yt = np.asarray(results[c]["yt"])  # [OT, P, N_SHARD] f32
        outs.append(yt.reshape(O, N_SHARD).T)  # [N_SHARD, O]
    return np.ascontiguousarray(np.concatenate(outs, axis=0)).astype(np.float32)


def kernel(x, weight, bias):
    from concourse.bass_utils import run_bass_kernel_spmd

    in_maps = prep_inputs(x, weight, bias)
    nc = get_nc()
    res = run_bass_kernel_spmd(nc, in_maps, list(range(N_CORES)))
    return gather_output(res.results)


# revision 9
# speedup vs baseline: 1.0301x; 1.0301x over previous
"""BinaryLinear (XNOR-Net style) Trainium2 kernel.

y = x @ (sign(W) * alpha)^T + bias,  alpha = mean(|W|, axis=1)

Strategy: data-parallel over the 16384-token dim across 8 NeuronCores.
Host folds the weight transform: signs are exactly representable in bf16,
so each core runs a bf16 matmul  y_shard^T[o, n] = sum_i sign(W)[o,i] *
x[n,i]  with fp32 PSUM accumulation, then applies the fp32 per-row scale
alpha[o] and bias[o] on the Scalar engine.  Host gathers/transposes back.
"""

import numpy as np
import ml_dtypes

N_CORES = 8
N_TOK = 16384
K = 4096  # in_features (contraction)
O = 4096  # out_features
P = 128
N_SHARD = N_TOK // N_CORES  # 2048 tokens per core
KO = K // P  # 32 contraction tiles
OT = O // P  # 32 output-feature tiles
NT = 512  # matmul moving free dim (one fp32 PSUM bank)
N_NT = N_SHARD // NT  # 4

_NC_CACHE = {}


def _build(n_shard=N_SHARD, ko=KO, ot=OT, nt=NT, st_dt="bfloat16", xt_dt="bfloat16"):
    import concourse.mybir as mybir
    import concourse.tile as tile
    from concourse import bacc

    st_dtype = getattr(mybir.dt, st_dt)
    xt_dtype = getattr(mybir.dt, xt_dt)
    f32 = mybir.dt.float32
    n_nt = n_shard // nt

    nc = bacc.Bacc("TRN2", target_bir_lowering=False, debug=False, num_devices=N_CORES)
    xt_d = nc.dram_tensor("xt", [ko, P, n_shard], xt_dtype, kind="ExternalInput")
    st_d = nc.dram_tensor("st", [ot, P, ko, P], st_dtype, kind="ExternalInput")
    al_d = nc.dram_tensor("alpha", [P, ot], f32, kind="ExternalInput")
    bi_d = nc.dram_tensor("bias", [P, ot], f32, kind="ExternalInput")
    yt_d = nc.dram_tensor("yt", [ot, P, n_shard], f32, kind="ExternalOutput")

    # warmup: first W o-tiles run k-major (k outer, 8 PSUM groups live) so the
    # PE starts as soon as each xt k-tile lands instead of waiting for the
    # whole resident x^T block.
    warm = max(1, min(8 // n_nt, ot))

    with tile.TileContext(nc) as tc:
        with (
            tc.tile_pool(name="xpool", bufs=1) as xpool,
            tc.tile_pool(name="spool", bufs=warm + 1) as spool,
            tc.tile_pool(name="opool", bufs=6) as opool,
            tc.tile_pool(name="cpool", bufs=1) as cpool,
            tc.tile_pool(name="psum", bufs=8, space="PSUM") as pp,
        ):
            al_t = cpool.tile([P, ot], f32)
            nc.sync.dma_start(al_t[:], al_d[:])
            bi_t = cpool.tile([P, ot], f32)
            nc.sync.dma_start(bi_t[:], bi_d[:])

            # x^T shard stays resident in SBUF: [128, ko, n_shard].
            xt_t = xpool.tile([P, ko, n_shard], xt_dtype)

            def epilogue(o, n, ps):
                ob = opool.tile([P, nt], f32)
                nc.scalar.activation(
                    ob[:],
                    ps[:],
                    mybir.ActivationFunctionType.Identity,
                    bias=bi_t[:, o : o + 1],
                    scale=al_t[:, o : o + 1],
                )
                nc.sync.dma_start(yt_d[o, :, n * nt : (n + 1) * nt], ob[:])

            # -- warmup phase: o-tiles [0, warm), k-major, DMAs k-sliced --
            s_ts = [
                spool.tile([P, ko, P], st_dtype, tag="s_t", name=f"s_w{o}")
                for o in range(warm)
            ]
            pss = [
                [
                    pp.tile([P, nt], f32, tag="ps", name=f"ps_w{o}_{n}")
                    for n in range(n_nt)
                ]
                for o in range(warm)
            ]
            for k in range(ko):
                nc.sync.dma_start(xt_t[:, k, :], xt_d[k])
                for o in range(warm):
                    nc.sync.dma_start(s_ts[o][:, k, :], st_d[o, :, k, :])
                for o in range(warm):
                    for n in range(n_nt):
                        nc.tensor.matmul(
                            pss[o][n][:],
                            s_ts[o][:, k, :],
                            xt_t[:, k, n * nt : (n + 1) * nt],
                            start=(k == 0),
                            stop=(k == ko - 1),
                        )
            for o in range(warm):
                for n in range(n_nt):
                    epilogue(o, n, pss[o][n])

            # -- steady phase --
            for o in range(warm, ot):
                s_t = spool.tile([P, ko, P], st_dtype, tag="s_t")
                nc.sync.dma_start(s_t[:], st_d[o])
                for n in range(n_nt):
                    ps = pp.tile([P, nt], f32, tag="ps")
                    for k in range(ko):
                        nc.tensor.matmul(
                            ps[:],
                            s_t[:, k, :],
                            xt_t[:, k, n * nt : (n + 1) * nt],
                            start=(k == 0),
                            stop=(k == ko - 1),
                        )
                    epilogue(o, n, ps)
    nc.compile()
    return nc


def _build_f32r(n_shard=N_SHARD, ko=KO, ot=OT, nt=NT, blk=1024):
    """float32r variant: x kept fp32 (f32r matmul, ~1 cyc/row at free>=256).

    x^T doesn't fit SBUF in fp32, so process n in blocks of `blk`.  Each block
    starts with a k-major warmup over the first W o-tiles (8 PSUM groups) so
    the PE runs while the x^T block streams in; warmup signs arrive as bf16
    k-slices (half the DMA) and are upcast on the Vector engine.
    """
    import concourse.mybir as mybir
    import concourse.tile as tile
    from concourse import bacc

    f32r = mybir.dt.float32r
    f32 = mybir.dt.float32
    bf16 = mybir.dt.bfloat16
    n_blocks = n_shard // blk
    n_nt = blk // nt  # psum groups per o-tile within a block
    W = max(1, min(8 // n_nt, ot))  # warmup o-tiles (W*n_nt = 8 banks)

    nc = bacc.Bacc("TRN2", target_bir_lowering=False, debug=False, num_devices=N_CORES)
    xt_d = nc.dram_tensor("xt", [ko, P, n_shard], f32r, kind="ExternalInput")
    st_d = nc.dram_tensor("st", [ot, P, ko, P], f32r, kind="ExternalInput")
    sw_d = nc.dram_tensor("sw", [ko, P, W, P], bf16, kind="ExternalInput")
    al_d = nc.dram_tensor("alpha", [P, ot], f32, kind="ExternalInput")
    bi_d = nc.dram_tensor("bias", [P, ot], f32, kind="ExternalInput")
    yt_d = nc.dram_tensor("yt", [ot, P, n_shard], f32, kind="ExternalOutput")

    with tile.TileContext(nc) as tc:
        with (
            tc.tile_pool(name="xpool", bufs=1) as xpool,
            tc.tile_pool(name="spool", bufs=2) as spool,
            tc.tile_pool(name="swbp", bufs=3) as swbp,
            tc.tile_pool(name="swfp", bufs=3) as swfp,
            tc.tile_pool(name="opool", bufs=4) as opool,
            tc.tile_pool(name="cpool", bufs=1) as cpool,
            tc.tile_pool(name="psum", bufs=8, space="PSUM") as pp,
        ):
            al_t = cpool.tile([P, ot], f32)
            nc.sync.dma_start(al_t[:], al_d[:])
            bi_t = cpool.tile([P, ot], f32)
            nc.sync.dma_start(bi_t[:], bi_d[:])

            def epilogue(o, gn, ps):
                ob = opool.tile([P, nt], f32, tag="ob", name=f"ob_{o}_{gn}")
                nc.scalar.activation(
                    ob[:],
                    ps[:],
                    mybir.ActivationFunctionType.Identity,
                    bias=bi_t[:, o : o + 1],
                    scale=al_t[:, o : o + 1],
                )
                nc.sync.dma_start(yt_d[o, :, gn * nt : (gn + 1) * nt], ob[:])

            for b in range(n_blocks):
                n0 = b * blk
                xt_t = xpool.tile([P, ko, blk], f32r, tag="xt", name=f"xt_b{b}")

                # -- warmup: o in [0, W), k-major, signs as bf16 k-slices --
                pss = [
                    [
                        pp.tile([P, nt], f32, tag="ps", name=f"ps_w{b}_{o}_{n}")
                        for n in range(n_nt)
                    ]
                    for o in range(W)
                ]
                for k in range(ko):
                    nc.sync.dma_start(xt_t[:, k, :], xt_d[k, :, n0 : n0 + blk])
                    swb_k = swbp.tile([P, W, P], bf16, tag="swb", name=f"swb_{b}_{k}")
                    nc.sync.dma_start(swb_k[:], sw_d[k])
                    swf_k = swfp.tile([P, W, P], f32r, tag="swf", name=f"swf_{b}_{k}")
                    nc.vector.tensor_copy(swf_k[:], swb_k[:])
                    for o in range(W):
                        for n in range(n_nt):
                            nc.tensor.matmul(
                                pss[o][n][:],
                                swf_k[:, o, :],
                                xt_t[:, k, n * nt : (n + 1) * nt],
                                start=(k == 0),
                                stop=(k == ko - 1),
                            )
                for o in range(W):
                    for n in range(n_nt):
                        epilogue(o, (n0 // nt) + n, pss[o][n])

                # -- steady: o in [W, ot), n-outer k-inner, signs fp32 --
                for o in range(W, ot):
                    s_t = spool.tile([P, ko, P], f32r, tag="s_t", name=f"s_{b}_{o}")
                    nc.sync.dma_start(s_t[:], st_d[o])
                    for n in range(n_nt):
                        ps = pp.tile([P, nt], f32, tag="ps", name=f"ps_{b}_{o}_{n}")
                        for k in range(ko):
                            nc.tensor.matmul(
                                ps[:],
                                s_t[:, k, :],
                                xt_t[:, k, n * nt : (n + 1) * nt],
                                start=(k == 0),
                                stop=(k == ko - 1),
                            )
                        epilogue(o, (n0 // nt) + n, ps)
    nc.compile()
    return nc


VARIANT = "f32r"  # "f32r" | "bf16"


def get_nc():
    key = f"nc_{VARIANT}"
    if key not in _NC_CACHE:
        _NC_CACHE[key] = _build_f32r() if VARIANT == "f32r" else _build()
    return _NC_CACHE[key]


def prep_inputs(x, weight, bias):
    """Host-side shard + layout prep. Returns in_maps for the 8 cores."""
    bf16 = ml_dtypes.bfloat16
    w = np.asarray(weight, dtype=np.float32)
    alpha = np.abs(w).mean(axis=1, dtype=np.float32).astype(np.float32)  # [O]
    s = np.sign(w).astype(bf16)  # [O, K], exactly +-1 (or 0)
    # layout (ot, p=k%128, ko, oi): 8KB contiguous per partition per o-tile
    st = np.ascontiguousarray(s.reshape(OT, P, KO, P).transpose(0, 3, 2, 1))
    al = np.ascontiguousarray(alpha.reshape(OT, P).T)
    bi = np.ascontiguousarray(np.asarray(bias, dtype=np.float32).reshape(OT, P).T)

    in_maps = []
    for c in range(N_CORES):
        xc = np.asarray(x[c * N_SHARD : (c + 1) * N_SHARD], dtype=np.float32)
        xt = np.ascontiguousarray(xc.T).astype(bf16).reshape(KO, P, N_SHARD)
        in_maps.append({"xt": xt, "st": st, "alpha": al, "bias": bi})
    return in_maps


def gather_output(results):
    outs = []
    for c in range(N_CORES):
        yt = np.asarray(results[c]["yt"])  # [OT, P, N_SHARD] f32
        outs.append(yt.reshape(O, N_SHARD).T)  # [N_SHARD, O]
    return np.ascontiguousarray(np.concatenate(outs, axis=0)).astype(np.float32)


def kernel(x, weight, bias):
    from concourse.bass_utils import run_bass_kernel_spmd

    in_maps = prep_inputs(x, weight, bias)
    nc = get_nc()
    res = run_bass_kernel_spmd(nc, in_maps, list(range(N_CORES)))
    return gather_output(res.results)
